# revision 30
# baseline (speedup 1.0000x reference)
"""Trainium2 Bass kernel for nn_LrFeatureUpScaler (2-layer TransformerConv GNN).

Sharding over 8 NeuronCores:
  conv1 (4 heads): core i = (head i//2, node-half i%2). Each core computes its
    head's k/v over all nodes, q/skip for its 512 target nodes, in transposed
    [feature, node] layout. One bf16 AllGather shares pre-norm h1 blocks +
    GraphNorm1 partial sums (f32 bit-cast); each core then normalizes full h1.
  conv2 (8 heads): core i = head i; fully local. GraphNorm2 local.
  Final row-normalize: [1024] f32 AllReduce of per-node partial sum-squares.

Perf notes (vs v1): PE is capped near 1.2 GHz (throttle), so the wins are
overlap/latency: constant-shift softmax (no reduce_max barrier), exp row-sums
via scalar accum_out, fused tensor_tensor_reduce, bf16 collective with merged
readback DMAs spread across engine queues, conv2 weight loads hoisted over the
collective, h2 transposes overlapped with the final AllReduce, batched stores.
"""
import numpy as np
import ml_dtypes

import concourse.bass as bass
import concourse.mybir as mybir
import concourse.tile as tile
from concourse.bass_utils import run_bass_kernel_spmd
from concourse.masks import make_identity

N = 1024
HR = 2048
EPS = 1e-5
INV_S = float(1.0 / np.sqrt(512.0))
SOFTMAX_SHIFT = -12.0  # logits are ~N(0,1); constant shift is overflow-safe
F32 = mybir.dt.float32
BF16 = mybir.dt.bfloat16
AF = mybir.ActivationFunctionType
ALU = mybir.AluOpType
AX = mybir.AxisListType
N_CORES = 8
AG_DATA = 512 * 512          # flat bf16 elements of h1 block data


def build_nc():
    nc = bass.Bass()
    # ---- I/O ----
    xT = nc.dram_tensor("xT", [N, N], BF16, kind="ExternalInput")
    xt_tgt = nc.dram_tensor("xt_tgt", [N, 512], BF16, kind="ExternalInput")
    x_edge = nc.dram_tensor("x_edge", [512, N], BF16, kind="ExternalInput")
    wq1 = nc.dram_tensor("wq1", [N, 512], BF16, kind="ExternalInput")
    wk1 = nc.dram_tensor("wk1", [N, 512], BF16, kind="ExternalInput")
    wv1 = nc.dram_tensor("wv1", [N, 512], BF16, kind="ExternalInput")
    ws1 = nc.dram_tensor("ws1", [N, 512], BF16, kind="ExternalInput")
    bq1c = nc.dram_tensor("bq1c", [128, 4], F32, kind="ExternalInput")
    bk1c = nc.dram_tensor("bk1c", [128, 4], F32, kind="ExternalInput")
    bvs1c = nc.dram_tensor("bvs1c", [128, 4], F32, kind="ExternalInput")
    we1cb = nc.dram_tensor("we1cb", [128, 4], BF16, kind="ExternalInput")
    we1rb = nc.dram_tensor("we1rb", [1, 512], BF16, kind="ExternalInput")
    gn1g = nc.dram_tensor("gn1g", [128, 16], F32, kind="ExternalInput")
    gn1b = nc.dram_tensor("gn1b", [128, 16], F32, kind="ExternalInput")
    gn1m = nc.dram_tensor("gn1m", [128, 16], F32, kind="ExternalInput")
    wq2 = nc.dram_tensor("wq2", [HR, 512], BF16, kind="ExternalInput")
    wk2 = nc.dram_tensor("wk2", [HR, 512], BF16, kind="ExternalInput")
    wv2 = nc.dram_tensor("wv2", [HR, 512], BF16, kind="ExternalInput")
    ws2 = nc.dram_tensor("ws2", [HR, 512], BF16, kind="ExternalInput")
    bq2c = nc.dram_tensor("bq2c", [128, 4], F32, kind="ExternalInput")
    bk2c = nc.dram_tensor("bk2c", [128, 4], F32, kind="ExternalInput")
    bvs2c = nc.dram_tensor("bvs2c", [128, 4], F32, kind="ExternalInput")
    we2cb = nc.dram_tensor("we2cb", [128, 4], BF16, kind="ExternalInput")
    we2rb = nc.dram_tensor("we2rb", [1, 512], BF16, kind="ExternalInput")
    gn2g = nc.dram_tensor("gn2g", [128, 4], F32, kind="ExternalInput")
    gn2b = nc.dram_tensor("gn2b", [128, 4], F32, kind="ExternalInput")
    gn2m = nc.dram_tensor("gn2m", [128, 4], F32, kind="ExternalInput")
    out = nc.dram_tensor("out", [N, 512], F32, kind="ExternalOutput")

    with tile.TileContext(nc) as tc:
        with (
            tc.tile_pool(name="const", bufs=1) as cp,
            tc.tile_pool(name="xp", bufs=1) as xp,
            tc.tile_pool(name="h1p", bufs=1) as h1p,
            tc.tile_pool(name="wc", bufs=28) as wc,
            tc.tile_pool(name="sm", bufs=2) as sm,
            tc.tile_pool(name="smc", bufs=1) as smc,
            tc.tile_pool(name="dram", bufs=1, space="DRAM") as dp,
            tc.tile_pool(name="pp", bufs=4, space="PSUM") as pp,
            tc.tile_pool(name="ppt", bufs=2, space="PSUM") as ppt,
            tc.tile_pool(name="pps", bufs=2, space="PSUM") as pps,
        ):
            # ---------- warmup collective (absorbs CC init/sync cost) ----------
            warm_in = dp.tile([64], F32, name="warm_in")
            warm_out = dp.tile([8, 64], F32, name="warm_out",
                               addr_space="Shared")
            nc.gpsimd.collective_compute(
                "AllGather", ALU.bypass,
                ins=[warm_in.opt()], outs=[warm_out.opt()],
                replica_groups=[list(range(N_CORES))],
            )

            # ---------- x / conv1-weight loads (critical path order) ----------
            def load_w(eng, wt, n_k, nm):
                ts_ = []
                for fc in range(n_k):
                    t = wc.tile([128, 512], BF16, name=f"{nm}{fc}", tag="wc")
                    eng.dma_start(t[:, :], wt[fc * 128:(fc + 1) * 128, :])
                    ts_.append(t)
                return ts_

            xt_sb = []
            for fc in range(8):
                t = xp.tile([128, 512], BF16, name=f"xt{fc}")
                nc.scalar.dma_start(t[:, :], xt_tgt[fc * 128:(fc + 1) * 128, :])
                xt_sb.append(t)
            wq_sb = load_w(nc.sync, wq1, 8, "wq1_")
            xT_sb = [None] * 8
            for fc in range(6):
                t = xp.tile([128, N], BF16, name=f"xT{fc}")
                nc.sync.dma_start(t[:, :], xT[fc * 128:(fc + 1) * 128, :])
                xT_sb[fc] = t
            wk_sb = load_w(nc.scalar, wk1, 8, "wk1_")
            xe_sb = []
            for cc in range(4):
                t = xp.tile([128, N], BF16, name=f"xe{cc}")
                nc.gpsimd.dma_start(t[:, :], x_edge[cc * 128:(cc + 1) * 128, :])
                xe_sb.append(t)
            for fc in range(6, 8):
                t = xp.tile([128, N], BF16, name=f"xT{fc}")
                nc.gpsimd.dma_start(t[:, :], xT[fc * 128:(fc + 1) * 128, :])
                xT_sb[fc] = t
            wv_sb = load_w(nc.gpsimd, wv1, 8, "wv1_")
            ws_sb = load_w(nc.sync, ws1, 8, "ws1_")

            # ---------- constants / small vectors ----------
            ident = cp.tile([128, 128], F32, name="ident")
            make_identity(nc, ident[:, :])
            ident_bf = cp.tile([128, 128], BF16, name="ident_bf")
            nc.vector.tensor_copy(ident_bf[:, :], ident[:, :])
            ones_col = cp.tile([128, 1], BF16, name="ones_col")
            nc.gpsimd.memset(ones_col[:, :], 1.0)
            eps_col = cp.tile([128, 1], F32, name="eps_col")
            nc.gpsimd.memset(eps_col[:, :], EPS)
            shift_col = cp.tile([128, 1], F32, name="shift_col")
            nc.gpsimd.memset(shift_col[:, :], SOFTMAX_SHIFT)

            def vload(t, shape, nm, dt=F32, eng=None):
                s = cp.tile(shape, dt, name=nm)
                (eng or nc.scalar).dma_start(s[:, :], t[:, :])
                return s

            bq1s = vload(bq1c, [128, 4], "bq1s")
            bk1s = vload(bk1c, [128, 4], "bk1s")
            bvs1s = vload(bvs1c, [128, 4], "bvs1s")
            we1cs = vload(we1cb, [128, 4], "we1cs", BF16)
            we1rs = vload(we1rb, [1, 512], "we1rs", BF16)
            gn1gs = vload(gn1g, [128, 16], "gn1gs", eng=nc.gpsimd)
            gn1bs = vload(gn1b, [128, 16], "gn1bs", eng=nc.gpsimd)
            gn1ms = vload(gn1m, [128, 16], "gn1ms", eng=nc.gpsimd)
            bq2s = vload(bq2c, [128, 4], "bq2s")
            bk2s = vload(bk2c, [128, 4], "bk2s")
            bvs2s = vload(bvs2c, [128, 4], "bvs2s")
            we2cs = vload(we2cb, [128, 4], "we2cs", BF16)
            we2rs = vload(we2rb, [1, 512], "we2rs", BF16)
            gn2gs = vload(gn2g, [128, 4], "gn2gs", eng=nc.gpsimd)
            gn2bs = vload(gn2b, [128, 4], "gn2bs", eng=nc.gpsimd)
            gn2ms = vload(gn2m, [128, 4], "gn2ms", eng=nc.gpsimd)

            h1T = [h1p.tile([128, N], BF16, name=f"h1T{f}") for f in range(16)]

            # DRAM collective buffers
            ag_inA = dp.tile([AG_DATA // 2 + 1024], BF16, name="ag_inA")
            ag_outA = dp.tile([8, AG_DATA // 2 + 1024], BF16, name="ag_outA",
                              addr_space="Shared")
            ag_inB = dp.tile([AG_DATA // 2 + 1024], BF16, name="ag_inB")
            ag_outB = dp.tile([8, AG_DATA // 2 + 1024], BF16, name="ag_outB",
                              addr_space="Shared")

            rn_in = dp.tile([1, N], F32, name="rn_in")
            rn_out = dp.tile([1, N], F32, name="rn_out", addr_space="Shared")

            copy_engines = [nc.vector, nc.scalar]

            def psum_copy(i, dst, src, bias=None):
                e = copy_engines[i % 2]
                if e is nc.vector:
                    if bias is None:
                        nc.vector.tensor_copy(dst, src)
                    else:
                        nc.vector.tensor_scalar(dst, src, bias, None, ALU.add)
                else:
                    if bias is None:
                        nc.scalar.activation(dst, src, AF.Copy)
                    else:
                        nc.scalar.activation(dst, src, AF.Identity, bias=bias)

            # shared softmax chunk: logits psum halves ps0/ps1, xe [128,1024]
            # bf16 edge rows, qe col [128,1]; writes alpha^T into aT tiles and
            # the edge-gate t into t_cols[:, tci].
            def softmax_chunk(cc, ps0, ps1, xe, qe_col, t_cols, tci, aT, pfx):
                ed = sm.tile([128, N], F32, name=f"{pfx}ed{cc}", tag="ed")
                nc.scalar.activation(ed[:, :], xe, AF.Copy, scale=qe_col)
                al = sm.tile([128, N], BF16, name=f"{pfx}al{cc}", tag="al")
                nc.vector.tensor_tensor(al[:, 0:512], ed[:, 0:512], ps0,
                                        ALU.add)
                nc.vector.tensor_tensor(al[:, 512:1024], ed[:, 512:1024], ps1,
                                        ALU.add)
                sc2 = smc.tile([128, 2], F32, name=f"{pfx}sc{cc}")
                nc.scalar.activation(al[:, 0:512], al[:, 0:512], AF.Exp,
                                     bias=shift_col[:, :], scale=INV_S,
                                     accum_out=sc2[:, 0:1])
                nc.scalar.activation(al[:, 512:1024], al[:, 512:1024], AF.Exp,
                                     bias=shift_col[:, :], scale=INV_S,
                                     accum_out=sc2[:, 1:2])
                scol = smc.tile([128, 1], F32, name=f"{pfx}s{cc}")
                nc.vector.tensor_tensor(scol[:, :], sc2[:, 0:1], sc2[:, 1:2],
                                        ALU.add)
                rcol = smc.tile([128, 1], F32, name=f"{pfx}r{cc}")
                nc.vector.reciprocal(rcol[:, :], scol[:, :])
                nc.vector.tensor_scalar_mul(al[:, :], al[:, :], rcol[:, :])
                for rc in range(8):
                    pst = ppt.tile([128, 128], BF16, name=f"{pfx}t{cc}{rc}",
                                   tag="tr")
                    nc.tensor.transpose(pst[:, :],
                                        al[:, rc * 128:(rc + 1) * 128],
                                        ident_bf[:, :])
                    nc.scalar.activation(aT[rc][:, cc * 128:(cc + 1) * 128],
                                         pst[:, :], AF.Copy)
                # edge gate t from normalized alpha (off the PE path)
                nc.vector.tensor_tensor(ed[:, :], al[:, :], xe, ALU.mult)
                nc.vector.reduce_sum(t_cols[:, tci:tci + 1], ed[:, :],
                                     axis=AX.X)

            # ================= CONV1 =================
            with tc.tile_pool(name="c1", bufs=1) as c1p:
                # q projection (my targets): qT [512d, 512c]
                qT = [c1p.tile([128, 512], BF16, name=f"qT{dc}")
                      for dc in range(4)]
                for dc in range(4):
                    ps = pp.tile([128, 512], F32, name=f"psq{dc}", tag="mm")
                    for fc in range(8):
                        nc.tensor.matmul(ps[:, :],
                                         wq_sb[fc][:, dc * 128:(dc + 1) * 128],
                                         xt_sb[fc][:, :], fc == 0, fc == 7)
                    psum_copy(dc, qT[dc][:, :], ps[:, :],
                              bias=bq1s[:, dc:dc + 1])
                # k projection (all nodes): kT [512d, 1024r]
                kT = [c1p.tile([128, N], BF16, name=f"kT{dc}")
                      for dc in range(4)]
                for dc in range(4):
                    for rh in range(2):
                        ps = pp.tile([128, 512], F32, name=f"psk{dc}{rh}",
                                     tag="mm")
                        for fc in range(8):
                            nc.tensor.matmul(
                                ps[:, :],
                                wk_sb[fc][:, dc * 128:(dc + 1) * 128],
                                xT_sb[fc][:, rh * 512:(rh + 1) * 512],
                                fc == 0, fc == 7)
                        psum_copy(dc * 2 + rh,
                                  kT[dc][:, rh * 512:(rh + 1) * 512], ps[:, :],
                                  bias=bk1s[:, dc:dc + 1])
                # v natural [1024n, 512d] (bias folded into output bias)
                v_bf = [c1p.tile([128, 512], BF16, name=f"v1_{nk}")
                        for nk in range(8)]
                for nk in range(8):
                    ps = pp.tile([128, 512], F32, name=f"psv{nk}", tag="mm")
                    for fc in range(8):
                        nc.tensor.matmul(ps[:, :],
                                         xT_sb[fc][:, nk * 128:(nk + 1) * 128],
                                         wv_sb[fc][:, :], fc == 0, fc == 7)
                    psum_copy(nk, v_bf[nk][:, :], ps[:, :])

                # qe[c] = q_c . We  (col layout [128,4])
                qe_cols = smc.tile([128, 4], F32, name="qe_cols")
                for cc in range(4):
                    psq = pps.tile([128, 1], F32, name=f"psqe{cc}", tag="sm")
                    for dc in range(4):
                        nc.tensor.matmul(psq[:, :],
                                         qT[dc][:, cc * 128:(cc + 1) * 128],
                                         we1cs[:, dc:dc + 1], dc == 0, dc == 3)
                    nc.scalar.activation(qe_cols[:, cc:cc + 1], psq[:, :],
                                         AF.Copy)

                # softmax + transpose, per 128-target chunk
                aT_bf = [c1p.tile([128, 512], BF16, name=f"aT1_{rc}")
                         for rc in range(8)]
                t1_cols = smc.tile([128, 4], F32, name="t1_cols")
                for cc in range(4):
                    ps0 = pp.tile([128, 512], F32, name=f"psa{cc}", tag="mm")
                    ps1 = pp.tile([128, 512], F32, name=f"psb{cc}", tag="mm")
                    for dc in range(4):
                        nc.tensor.matmul(ps0[:, :],
                                         qT[dc][:, cc * 128:(cc + 1) * 128],
                                         kT[dc][:, 0:512], dc == 0, dc == 3)
                    for dc in range(4):
                        nc.tensor.matmul(ps1[:, :],
                                         qT[dc][:, cc * 128:(cc + 1) * 128],
                                         kT[dc][:, 512:1024], dc == 0, dc == 3)
                    softmax_chunk(cc, ps0[:, :], ps1[:, :], xe_sb[cc][:, :],
                                  qe_cols[:, cc:cc + 1], t1_cols, cc, aT_bf,
                                  "c1")
                # t row [1, 512]
                t1r = smc.tile([1, 512], BF16, name="t1r")
                pstr = pps.tile([1, 512], F32, name="pst1r", tag="sm")
                for cc in range(4):
                    nc.tensor.transpose(pstr[0:1, cc * 128:(cc + 1) * 128],
                                        t1_cols[:, cc:cc + 1], ident[:, :])
                nc.scalar.activation(t1r[0:1, :], pstr[0:1, :], AF.Copy)

                # output accumulation -> h1 block [512f, 512c] (bf16) + stats
                S12A = smc.tile([128, 4], F32, name="S12A")
                S12B = smc.tile([128, 4], F32, name="S12B")
                h1blk = [c1p.tile([128, 512], BF16, name=f"h1b{dc}")
                         for dc in range(4)]
                for dc in range(4):
                    S1S2 = S12A if dc < 2 else S12B
                    sci = dc % 2
                    ps = pp.tile([128, 512], F32, name=f"pso{dc}", tag="mm")
                    for rc in range(8):
                        nc.tensor.matmul(ps[:, :],
                                         v_bf[rc][:, dc * 128:(dc + 1) * 128],
                                         aT_bf[rc][:, :], rc == 0, False)
                    nc.tensor.matmul(ps[:, :],
                                     we1rs[0:1, dc * 128:(dc + 1) * 128],
                                     t1r[0:1, :], False, False)
                    for fc in range(8):
                        nc.tensor.matmul(ps[:, :],
                                         ws_sb[fc][:, dc * 128:(dc + 1) * 128],
                                         xt_sb[fc][:, :], False, fc == 7)
                    nc.vector.tensor_scalar(h1blk[dc][:, :], ps[:, :],
                                            bvs1s[:, dc:dc + 1], None, ALU.add)
                    # GraphNorm1 partials from full-precision psum
                    nc.vector.reduce_sum(S1S2[:, sci:sci + 1], ps[:, :],
                                         axis=AX.X)
                    sq = sm.tile([128, 512], BF16, name=f"sq1_{dc}", tag="sq")
                    nc.scalar.activation(sq[:, :], ps[:, :], AF.Square,
                                         accum_out=S1S2[:, 2 + sci:3 + sci])
                    agt = ag_inA if dc < 2 else ag_inB
                    off = (dc % 2) * 65536
                    nc.sync.dma_start(
                        agt[off:off + 65536].rearrange(
                            "(p c) -> p c", p=128), h1blk[dc][:, :])
                    if dc == 1:
                        nc.sync.dma_start(
                            ag_inA[131072:132096].rearrange(
                                "(p a) -> p a", p=128),
                            S12A[:, :].bitcast(BF16))
                    elif dc == 3:
                        nc.sync.dma_start(
                            ag_inB[131072:132096].rearrange(
                                "(p a) -> p a", p=128),
                            S12B[:, :].bitcast(BF16))

            # hoist conv2 weight loads over the collective (scalar/vector/
            # tensor/sync queues; keep gpsimd free for the trigger)
            def load_w2(eng, wt, nm):
                ts_ = []
                for fc in range(16):
                    t = wc.tile([128, 512], BF16, name=f"{nm}{fc}", tag="wc")
                    eng.dma_start(t[:, :], wt[fc * 128:(fc + 1) * 128, :])
                    ts_.append(t)
                return ts_

            wq2_sb = load_w2(nc.scalar, wq2, "wq2_")

            # h1 data (+stats bitcast, riding half A) in two halves so
            # conv2 can start on half A while B is still in flight
            nc.gpsimd.collective_compute(
                "AllGather", ALU.bypass,
                ins=[ag_inA.opt()], outs=[ag_outA.opt()],
                replica_groups=[list(range(N_CORES))],
            )
            nc.gpsimd.collective_compute(
                "AllGather", ALU.bypass,
                ins=[ag_inB.opt()], outs=[ag_outB.opt()],
                replica_groups=[list(range(N_CORES))],
            )

            # ---------- GraphNorm1 coeffs from stats (during big xfer) ----------
            rd_engines = [nc.sync, nc.scalar]

            def read_half(half):
                agt = ag_outA if half == 0 else ag_outB
                for hj in range(4):
                    for dci in range(2):
                        dc = half * 2 + dci
                        off = dci * 65536
                        for gg in range(2):
                            blk = agt[2 * hj + gg,
                                      off:off + 65536].rearrange(
                                          "(p c) -> p c", p=128)
                            rd_engines[(hj * 2 + dci + gg) % 2].dma_start(
                                h1T[hj * 4 + dc][:, gg * 512:(gg + 1) * 512],
                                blk)

            def read_stats(agt, nm):
                sst = []
                for j in range(8):
                    t = smc.tile([128, 8], BF16, name=f"sst{nm}{j}")
                    nc.gpsimd.dma_start(
                        t[:, :],
                        agt[j, 131072:132096].rearrange("(p a) -> p a",
                                                        p=128))
                    sst.append(t)
                return sst

            read_half(0)
            sstA = read_stats(ag_outA, "A")

            def half_sums(sst, nm):
                S1h = smc.tile([128, 8], F32, name=f"S1{nm}")
                S2h = smc.tile([128, 8], F32, name=f"S2{nm}")
                for h in range(4):
                    a0 = sst[2 * h][:, :].bitcast(F32)
                    a1 = sst[2 * h + 1][:, :].bitcast(F32)
                    nc.vector.tensor_tensor(S1h[:, 2 * h:2 * h + 2],
                                            a0[:, 0:2], a1[:, 0:2], ALU.add)
                    nc.vector.tensor_tensor(S2h[:, 2 * h:2 * h + 2],
                                            a0[:, 2:4], a1[:, 2:4], ALU.add)
                return S1h, S2h

            S1aA, S2aA = half_sums(sstA, "aA")

            def gnorm_coeffs(S1t, S2t, gc, bc, mc, w, nm):
                mu = smc.tile([128, w], F32, name=f"mu{nm}")
                nc.vector.tensor_scalar_mul(mu[:, :], S1t[:, :], 1.0 / N)
                ex2 = smc.tile([128, w], F32, name=f"ex2{nm}")
                nc.vector.tensor_scalar_mul(ex2[:, :], S2t[:, :], 1.0 / N)
                msmu = smc.tile([128, w], F32, name=f"msmu{nm}")
                nc.vector.tensor_tensor(msmu[:, :], mc[:, :], mu[:, :],
                                        ALU.mult)
                tmp = smc.tile([128, w], F32, name=f"tmp{nm}")
                nc.vector.tensor_scalar_mul(tmp[:, :], mu[:, :], 2.0)
                nc.vector.tensor_tensor(tmp[:, :], tmp[:, :], msmu[:, :],
                                        ALU.subtract)
                nc.vector.tensor_tensor(tmp[:, :], msmu[:, :], tmp[:, :],
                                        ALU.mult)
                var = smc.tile([128, w], F32, name=f"var{nm}")
                nc.vector.tensor_tensor(var[:, :], ex2[:, :], tmp[:, :],
                                        ALU.subtract)
                nc.scalar.activation(var[:, :], var[:, :], AF.Sqrt,
                                     bias=eps_col[:, :])
                rstd = smc.tile([128, w], F32, name=f"rstd{nm}")
                nc.vector.reciprocal(rstd[:, :], var[:, :])
                scl = smc.tile([128, w], F32, name=f"scl{nm}")
                nc.vector.tensor_tensor(scl[:, :], gc[:, :], rstd[:, :],
                                        ALU.mult)
                sh = smc.tile([128, w], F32, name=f"sh{nm}")
                nc.vector.tensor_tensor(sh[:, :], scl[:, :], msmu[:, :],
                                        ALU.mult)
                nc.vector.tensor_tensor(sh[:, :], bc[:, :], sh[:, :],
                                        ALU.subtract)
                return scl, sh

            # gn vectors are shipped half-major: col half*8 + hj*2 + dci
            sclA, shA = gnorm_coeffs(S1aA, S2aA, gn1gs[:, 0:8], gn1bs[:, 0:8],
                                     gn1ms[:, 0:8], 8, "g1a")

            def apply_half(scl, sh, half):
                for r_i in range(8):
                    hj, dci = r_i // 2, r_i % 2
                    f = hj * 4 + half * 2 + dci
                    if r_i % 2 == 0:
                        nc.vector.tensor_scalar(h1T[f][:, :], h1T[f][:, :],
                                                scl[:, r_i:r_i + 1],
                                                sh[:, r_i:r_i + 1],
                                                ALU.mult, ALU.add)
                    else:
                        nc.scalar.activation(h1T[f][:, :], h1T[f][:, :],
                                             AF.Identity,
                                             bias=sh[:, r_i:r_i + 1],
                                             scale=scl[:, r_i:r_i + 1])

            apply_half(sclA, shA, 0)

            # ================= CONV2 =================
            with tc.tile_pool(name="c2", bufs=1) as c2p:
                q2T = [c2p.tile([128, N], BF16, name=f"q2T{dc}")
                       for dc in range(4)]
                # half-A h1 tiles (dc 0,1 of each head) arrive first; ALL
                # eight q2 chains run their A-halves into SBUF partial sums
                # (bias pre-added) while B is still in flight, then finish
                # with the B-half and one vector add each
                fcA = [hj * 4 + dci for hj in range(4) for dci in (0, 1)]
                fcB = [hj * 4 + 2 + dci for hj in range(4) for dci in (0, 1)]
                qpart = [c2p.tile([128, 512], BF16, name=f"qp{i}")
                         for i in range(8)]
                for dc in range(4):
                    for ch in range(2):
                        ps = pp.tile([128, 512], F32, name=f"ps2qa{dc}{ch}",
                                     tag="mm")
                        for k, fc in enumerate(fcA):
                            nc.tensor.matmul(
                                ps[:, :],
                                wq2_sb[fc][:, dc * 128:(dc + 1) * 128],
                                h1T[fc][:, ch * 512:(ch + 1) * 512],
                                k == 0, k == 7)
                        psum_copy(dc * 2 + ch, qpart[dc * 2 + ch][:, :],
                                  ps[:, :], bias=bq2s[:, dc:dc + 1])
                # B-half pipeline while the PE chews on the A-pass.
                # tile_wait_until keeps the scheduler from hoisting these
                # (they block on the B collective) ahead of the A-critical
                # chain on shared engine queues.
                with tc.tile_wait_until(0.3):
                    read_half(1)
                    sstB = read_stats(ag_outB, "B")
                    S1aB, S2aB = half_sums(sstB, "aB")
                    sclB, shB = gnorm_coeffs(S1aB, S2aB, gn1gs[:, 8:16],
                                             gn1bs[:, 8:16], gn1ms[:, 8:16],
                                             8, "g1b")
                    apply_half(sclB, shB, 1)
                    wk2_sb = load_w2(nc.scalar, wk2, "wk2_")
                    wv2_sb = load_w2(nc.sync, wv2, "wv2_")
                    ws2_sb = load_w2(nc.gpsimd, ws2, "ws2_")
                for dc in range(4):
                    for ch in range(2):
                        ps = pp.tile([128, 512], F32, name=f"ps2qb{dc}{ch}",
                                     tag="mm")
                        for k, fc in enumerate(fcB):
                            nc.tensor.matmul(
                                ps[:, :],
                                wq2_sb[fc][:, dc * 128:(dc + 1) * 128],
                                h1T[fc][:, ch * 512:(ch + 1) * 512],
                                k == 0, k == 7)
                        nc.vector.tensor_tensor(
                            q2T[dc][:, ch * 512:(ch + 1) * 512], ps[:, :],
                            qpart[dc * 2 + ch][:, :], ALU.add)
                k2T = [c2p.tile([128, N], BF16, name=f"k2T{dc}")
                       for dc in range(4)]
                for dc in range(4):
                    for ch in range(2):
                        ps = pp.tile([128, 512], F32, name=f"ps2k{dc}{ch}",
                                     tag="mm")
                        for fc in range(16):
                            nc.tensor.matmul(
                                ps[:, :],
                                wk2_sb[fc][:, dc * 128:(dc + 1) * 128],
                                h1T[fc][:, ch * 512:(ch + 1) * 512],
                                fc == 0, fc == 15)
                        psum_copy(dc * 2 + ch + 1,
                                  k2T[dc][:, ch * 512:(ch + 1) * 512],
                                  ps[:, :], bias=bk2s[:, dc:dc + 1])
                v2_bf = [c2p.tile([128, 512], BF16, name=f"v2_{nk}")
                         for nk in range(8)]
                for nk in range(8):
                    ps = pp.tile([128, 512], F32, name=f"ps2v{nk}", tag="mm")
                    for fc in range(16):
                        nc.tensor.matmul(ps[:, :],
                                         h1T[fc][:, nk * 128:(nk + 1) * 128],
                                         wv2_sb[fc][:, :], fc == 0, fc == 15)
                    psum_copy(nk, v2_bf[nk][:, :], ps[:, :])

                qe2 = smc.tile([128, 8], F32, name="qe2")
                for cc in range(8):
                    psq = pps.tile([128, 1], F32, name=f"ps2e{cc}", tag="sm")
                    for dc in range(4):
                        nc.tensor.matmul(psq[:, :],
                                         q2T[dc][:, cc * 128:(cc + 1) * 128],
                                         we2cs[:, dc:dc + 1], dc == 0, dc == 3)
                    nc.scalar.activation(qe2[:, cc:cc + 1], psq[:, :], AF.Copy)

                aT2 = [c2p.tile([128, N], BF16, name=f"aT2_{rc}")
                       for rc in range(8)]
                t2_cols = smc.tile([128, 8], F32, name="t2_cols")
                for cc in range(8):
                    ps0 = pp.tile([128, 512], F32, name=f"p2a{cc}", tag="mm")
                    ps1 = pp.tile([128, 512], F32, name=f"p2b{cc}", tag="mm")
                    for dc in range(4):
                        nc.tensor.matmul(ps0[:, :],
                                         q2T[dc][:, cc * 128:(cc + 1) * 128],
                                         k2T[dc][:, 0:512], dc == 0, dc == 3)
                    for dc in range(4):
                        nc.tensor.matmul(ps1[:, :],
                                         q2T[dc][:, cc * 128:(cc + 1) * 128],
                                         k2T[dc][:, 512:1024], dc == 0,
                                         dc == 3)
                    softmax_chunk(cc, ps0[:, :], ps1[:, :], xT_sb[cc][:, :],
                                  qe2[:, cc:cc + 1], t2_cols, cc, aT2, "c2")
                t2r = smc.tile([1, N], BF16, name="t2r")
                for nh in range(2):
                    pstr = pps.tile([1, 512], F32, name=f"pst2r{nh}", tag="sm")
                    for cc in range(4):
                        nc.tensor.transpose(pstr[0:1, cc * 128:(cc + 1) * 128],
                                            t2_cols[:,
                                                    nh * 4 + cc:nh * 4 + cc + 1],
                                            ident[:, :])
                    nc.scalar.activation(t2r[0:1, nh * 512:(nh + 1) * 512],
                                         pstr[0:1, :], AF.Copy)

                h2T = [c2p.tile([128, N], F32, name=f"h2T{dc}")
                       for dc in range(4)]
                for dc in range(4):
                    for ch in range(2):
                        ps = pp.tile([128, 512], F32, name=f"ps2o{dc}{ch}",
                                     tag="mm")
                        for rc in range(8):
                            nc.tensor.matmul(
                                ps[:, :],
                                v2_bf[rc][:, dc * 128:(dc + 1) * 128],
                                aT2[rc][:, ch * 512:(ch + 1) * 512],
                                rc == 0, False)
                        nc.tensor.matmul(ps[:, :],
                                         we2rs[0:1, dc * 128:(dc + 1) * 128],
                                         t2r[0:1, ch * 512:(ch + 1) * 512],
                                         False, False)
                        for fc in range(16):
                            nc.tensor.matmul(
                                ps[:, :],
                                ws2_sb[fc][:, dc * 128:(dc + 1) * 128],
                                h1T[fc][:, ch * 512:(ch + 1) * 512],
                                False, fc == 15)
                        nc.vector.tensor_scalar(
                            h2T[dc][:, ch * 512:(ch + 1) * 512], ps[:, :],
                            bvs2s[:, dc:dc + 1], None, ALU.add)

                # GraphNorm2 (local)
                T1 = smc.tile([128, 4], F32, name="T1")
                T2 = smc.tile([128, 4], F32, name="T2")
                for dc in range(4):
                    nc.vector.reduce_sum(T1[:, dc:dc + 1], h2T[dc][:, :],
                                         axis=AX.X)
                    sq = sm.tile([128, N], BF16, name=f"sq2_{dc}", tag="al")
                    nc.scalar.activation(sq[:, :], h2T[dc][:, :], AF.Square,
                                         accum_out=T2[:, dc:dc + 1])
                scl2, sh2 = gnorm_coeffs(T1, T2, gn2gs, gn2bs, gn2ms, 4, "g2")
                for dc in range(4):
                    if dc % 2 == 0:
                        nc.vector.tensor_scalar(h2T[dc][:, :], h2T[dc][:, :],
                                                scl2[:, dc:dc + 1],
                                                sh2[:, dc:dc + 1],
                                                ALU.mult, ALU.add)
                    else:
                        nc.scalar.activation(h2T[dc][:, :], h2T[dc][:, :],
                                             AF.Identity,
                                             bias=sh2[:, dc:dc + 1],
                                             scale=scl2[:, dc:dc + 1])

                # row-norm partial sumsq (over my 512 features) via ones-matmul
                rn_row = smc.tile([1, N], F32, name="rn_row")
                for nh in range(2):
                    psr = pps.tile([1, 512], F32, name=f"psrn{nh}", tag="sm")
                    for dc in range(4):
                        sqh = sm.tile([128, 512], BF16, name=f"sqh{nh}{dc}",
                                      tag="sqh")
                        nc.scalar.activation(sqh[:, :],
                                             h2T[dc][:,
                                                     nh * 512:(nh + 1) * 512],
                                             AF.Square)
                        nc.tensor.matmul(psr[0:1, :], ones_col[:, :],
                                         sqh[:, :], dc == 0, dc == 3)
                    nc.scalar.activation(rn_row[0:1, nh * 512:(nh + 1) * 512],
                                         psr[0:1, :], AF.Copy)
                nc.sync.dma_start(rn_in[0:1, :], rn_row[0:1, :])

                nc.gpsimd.collective_compute(
                    "AllReduce", ALU.add,
                    ins=[rn_in.opt()], outs=[rn_out.opt()],
                    replica_groups=[list(range(N_CORES))],
                )

                # during the AllReduce: transpose h2 -> [node, feat] with
                # 4-deep psum pipelining, unscaled copies on vector+scalar
                # (PE busy keeps clocks up)
                obuf = [c2p.tile([128, 512], F32, name=f"ob{nk}")
                        for nk in range(8)]
                for nk in range(8):
                    for dc in range(4):
                        pst = pp.tile([128, 128], F32, name=f"pf{nk}{dc}",
                                      tag="mm")
                        nc.tensor.transpose(pst[:, :],
                                            h2T[dc][:,
                                                    nk * 128:(nk + 1) * 128],
                                            ident[:, :])
                        psum_copy(nk + dc,
                                  obuf[nk][:, dc * 128:(dc + 1) * 128],
                                  pst[:, :])
                # keep the PE warm through the AR wait (one accumulation
                # chain; serial by construction)
                pstW = ppt.tile([128, 128], F32, name="warmch", tag="tr")
                for w in range(88):
                    nc.tensor.matmul(pstW[:, :], ident_bf[:, :],
                                     ident_bf[:, :], w == 0, w == 39)

                # inv = 1/sqrt(total sumsq) per node, via on-chip transpose
                rint = smc.tile([1, N], F32, name="rint")
                nc.scalar.dma_start(rint[0:1, :], rn_out[0:1, :])
                tot8 = smc.tile([128, 8], F32, name="tot8")
                for nk in range(8):
                    # transpose trick in reverse: [1,128] -> [128,1]
                    pstI = ppt.tile([128, 128], F32, name=f"pi{nk}", tag="tr")
                    nc.tensor.matmul(
                        pstI[:, 0:1],
                        rint[0:1, nk * 128:(nk + 1) * 128],
                        ident[0:1, 0:1], True, True)
                    nc.scalar.activation(tot8[:, nk:nk + 1], pstI[:, 0:1],
                                         AF.Copy)
                nc.scalar.activation(tot8[:, :], tot8[:, :], AF.Sqrt)
                inv8 = smc.tile([128, 8], F32, name="inv8")
                nc.vector.reciprocal(inv8[:, :], tot8[:, :])

                # post-AR: in-place scale on scalar + store; dummy PE
                # transposes hold the clock domain up until stores are fed
                st_engines = [nc.sync, nc.gpsimd]
                for nk in range(8):
                    nc.scalar.activation(obuf[nk][:, :], obuf[nk][:, :],
                                         AF.Copy, scale=inv8[:, nk:nk + 1])
                    pstW = ppt.tile([128, 128], F32, name=f"warmb{nk}",
                                    tag="tr")
                    nc.tensor.transpose(pstW[:, :], ident[:, :], ident[:, :])
                    st_engines[nk % 2].dma_start(
                        out[nk * 128:(nk + 1) * 128, :], obuf[nk][:, :])
    return nc


_NC_CACHE = None


def _get_nc():
    global _NC_CACHE
    if _NC_CACHE is None:
        nc = build_nc()
        # local walrus only accepts one sync-wait per CTRL-class instruction
        for f in nc.m.functions:
            for bb in f.blocks:
                changed = False
                new_list = []
                for ins in bb.instructions:
                    si = ins.sync_info
                    if si is not None and len(si.on_wait) > 1:
                        waits = list(si.on_wait)
                        for i, w in enumerate(waits[:-1]):
                            nop = mybir.InstNoOp(
                                name=f"{ins.name}_presplit{i}",
                                engine=ins.engine)
                            nop.sync_info = mybir.SyncInfo(on_wait=[w],
                                                           on_update=[])
                            new_list.append(nop)
                        ins.sync_info = mybir.SyncInfo(
                            on_wait=[waits[-1]], on_update=list(si.on_update))
                        changed = True
                    new_list.append(ins)
                if changed:
                    bb.instructions = new_list
        _NC_CACHE = nc
    return _NC_CACHE


def _colmaj(v, w):
    """[w*128] vector -> [128, w] with [p, a] = v[a*128 + p]."""
    return np.ascontiguousarray(np.asarray(v, np.float32).reshape(w, 128).T)


_HM_PERM = [0, 1, 4, 5, 8, 9, 12, 13, 2, 3, 6, 7, 10, 11, 14, 15]


def _colmaj_hm(v):
    """[2048] -> [128, 16] col-major with columns in half-major tile order."""
    m = np.asarray(v, np.float32).reshape(16, 128)[_HM_PERM]
    return np.ascontiguousarray(m.T)


def _build_in_maps(inputs):
    x = np.asarray(inputs["x"], np.float32)
    bf = ml_dtypes.bfloat16

    def c(a, dt=np.float32):
        return np.ascontiguousarray(a).astype(dt)

    xT = np.ascontiguousarray(x.T)
    in_maps = []
    for i in range(N_CORES):
        h, g = i // 2, i % 2
        s1, s2i = slice(512 * h, 512 * (h + 1)), slice(512 * i, 512 * (i + 1))
        we1h = np.asarray(inputs["e1_w"], np.float32).reshape(4, 512)[h]
        we2h = np.asarray(inputs["e2_w"], np.float32).reshape(8, 512)[i]
        m = {
            "xT": c(xT, bf),
            "xt_tgt": c(xT[:, 512 * g:512 * (g + 1)], bf),
            "x_edge": c(xT[512 * g:512 * (g + 1), :], bf),
            "wq1": c(inputs["q1_w"][:, s1], bf),
            "wk1": c(inputs["k1_w"][:, s1], bf),
            "wv1": c(inputs["v1_w"][:, s1], bf),
            "ws1": c(inputs["s1_w"][:, s1], bf),
            "bq1c": _colmaj(inputs["q1_b"][s1], 4),
            "bk1c": _colmaj(inputs["k1_b"][s1], 4),
            "bvs1c": _colmaj(np.asarray(inputs["v1_b"][s1], np.float32)
                             + np.asarray(inputs["s1_b"][s1], np.float32), 4),
            "we1cb": c(_colmaj(we1h, 4), bf),
            "we1rb": c(we1h.reshape(1, 512), bf),
            "gn1g": _colmaj_hm(inputs["gn1_gamma"]),
            "gn1b": _colmaj_hm(inputs["gn1_beta"]),
            "gn1m": _colmaj_hm(inputs["gn1_ms"]),
            "wq2": c(inputs["q2_w"][:, s2i], bf),
            "wk2": c(inputs["k2_w"][:, s2i], bf),
            "wv2": c(inputs["v2_w"][:, s2i], bf),
            "ws2": c(inputs["s2_w"][:, s2i], bf),
            "bq2c": _colmaj(inputs["q2_b"][s2i], 4),
            "bk2c": _colmaj(inputs["k2_b"][s2i], 4),
            "bvs2c": _colmaj(np.asarray(inputs["v2_b"][s2i], np.float32)
                             + np.asarray(inputs["s2_b"][s2i], np.float32), 4),
            "we2cb": c(_colmaj(we2h, 4), bf),
            "we2rb": c(we2h.reshape(1, 512), bf),
            "gn2g": _colmaj(inputs["gn2_gamma"][s2i], 4),
            "gn2b": _colmaj(inputs["gn2_beta"][s2i], 4),
            "gn2m": _colmaj(inputs["gn2_ms"][s2i], 4),
        }
        in_maps.append(m)
    return in_maps


def kernel(**inputs):
    in_maps = _build_in_maps(inputs)
    res = _run_cached(in_maps)
    full = np.empty((N, 2 * HR), np.float32)
    for i in range(N_CORES):
        full[:, 512 * i:512 * (i + 1)] = res[i]["out"]
    return full


_RUNNER = None


def _get_runner():
    """Build the sharded jitted executable once per process."""
    global _RUNNER
    if _RUNNER is not None:
        return _RUNNER
    import jax
    from jax.sharding import Mesh, PartitionSpec, NamedSharding
    from jax.experimental.shard_map import shard_map
    from concourse import bass2jax
    from concourse.bass2jax import _bass_exec_p, install_neuronx_cc_hook

    nc = _get_nc()
    install_neuronx_cc_hook()
    partition_name = nc.partition_id_tensor.name if nc.partition_id_tensor else None
    in_names, out_names, out_avals, zero_outs = [], [], [], []
    for alloc in nc.m.functions[0].allocations:
        if not isinstance(alloc, mybir.MemoryLocationSet):
            continue
        name = alloc.memorylocations[0].name
        if alloc.kind == "ExternalInput":
            if name != partition_name:
                in_names.append(name)
        elif alloc.kind == "ExternalOutput":
            out_names.append(name)
            out_avals.append(jax.core.ShapedArray(
                tuple(alloc.tensor_shape), mybir.dt.np(alloc.dtype)))
            zero_outs.append(np.zeros(tuple(alloc.tensor_shape),
                                      mybir.dt.np(alloc.dtype)))
    n_params, n_outs = len(in_names), len(out_avals)
    all_names = in_names + out_names + ([partition_name] if partition_name else [])
    donate = tuple(range(n_params, n_params + n_outs))

    def _body(*args):
        operands = list(args)
        if partition_name is not None:
            operands.append(bass2jax.partition_id_tensor())
        return tuple(_bass_exec_p.bind(
            *operands, out_avals=tuple(out_avals), in_names=tuple(all_names),
            out_names=tuple(out_names), lowering_input_output_aliases=(),
            sim_require_finite=True, sim_require_nnan=True, nc=nc))

    devices = jax.devices()[:N_CORES]
    mesh = Mesh(np.asarray(devices), ("core",))
    sharded = jax.jit(
        shard_map(_body, mesh=mesh,
                  in_specs=(PartitionSpec("core"),) * (n_params + n_outs),
                  out_specs=(PartitionSpec("core"),) * n_outs,
                  check_rep=False),
        donate_argnums=donate, keep_unused=True)
    sh = NamedSharding(mesh, PartitionSpec("core"))
    _RUNNER = (sharded, sh, in_names, out_names, out_avals, zero_outs, jax)
    return _RUNNER


def _run_cached(in_maps):
    sharded, sh, in_names, out_names, out_avals, zero_outs, jax = _get_runner()
    concat_in = [np.concatenate([np.asarray(in_maps[c][nm])
                                 for c in range(N_CORES)], axis=0)
                 for nm in in_names]
    dev_in = [jax.device_put(a, sh) for a in concat_in]
    zs = [jax.device_put(np.zeros((N_CORES * z.shape[0], *z.shape[1:]), z.dtype), sh)
          for z in zero_outs]
    outs = sharded(*dev_in, *zs)
    outs = [np.asarray(o).reshape(N_CORES, *out_avals[i].shape)
            for i, o in enumerate(outs)]
    return [{nm: outs[i][c] for i, nm in enumerate(out_names)}
            for c in range(N_CORES)]
